# revision 1
# baseline (speedup 1.0000x reference)
"""Causal attention kernel for trn2, sharded over 8 NeuronCores.

Problem (B=4, S=2048, E=2048, H=16, D=128), fp32:
    qkv = x @ w_qkv; q,k,v = split(qkv)
    q,k,v reshaped (B,S,E)->(B,H,S,D) as a RAW view (no transpose), i.e.
    per (b,h): Q_h = rows [h*128,(h+1)*128) of q[b] reinterpreted [S,D].
    o = softmax(QK^T/sqrt(D) + causal(+1/-10000)) @ V, inverse raw view,
    out = o @ w_out.

Because the raw view maps head h to a contiguous block of 128 sequence
rows, the whole computation splits into B*H = 64 independent tasks, each
touching only x[b, h*128:(h+1)*128, :] and producing
out[b, h*128:(h+1)*128, :].  Core c gets 8 tasks = rows
[c*1024,(c+1)*1024) of x.reshape(B*S, E).  No collectives.
"""

import numpy as np

B, S, E = 4, 2048, 2048
H, D, P = 16, 128, 128
NCORES = 8
ROWS = B * S // NCORES  # 1024 rows per core = 8 tasks of 128 rows
SCALE = float(1.0 / np.sqrt(D))
NEG = -1.0e9  # pre-scale additive mask; exp underflows to exactly 0.0

_NC_CACHE = {}


def build_nc(task_ids=tuple(range(8)), mm="float32r", group_size=4,
             iters=1):
    import concourse.bass as bass
    import concourse.mybir as mybir
    import concourse.tile as tile
    from concourse import bacc
    from concourse.masks import make_identity

    f32 = mybir.dt.float32
    mm_dt = getattr(mybir.dt, mm)
    AF = mybir.ActivationFunctionType
    ALU = mybir.AluOpType

    def wview(ap):  # DRAM weight view in matmul dtype (bit-identical)
        return ap.bitcast(mm_dt) if mm_dt != f32 else ap

    nc = bacc.Bacc("TRN2", target_bir_lowering=False, debug=False,
                   num_devices=NCORES)
    xs = nc.dram_tensor("xs", [ROWS, E], f32, kind="ExternalInput")
    wqkv = nc.dram_tensor("wqkv", [E, 3 * E], f32, kind="ExternalInput")
    wout = nc.dram_tensor("wout", [E, E], f32, kind="ExternalInput")
    out = nc.dram_tensor("out", [ROWS, E], f32, kind="ExternalOutput")

    # DRAM views: partition = kk within 128-row blocks of the contraction dim
    wqkv_v = wqkv.ap().rearrange("(ko p) c -> p ko c", p=P)   # [128,16,6144]
    wout_v = wout.ap().rearrange("(co p) n -> p co n", p=P)   # [128,16,2048]

    groups = [list(task_ids)[i:i + group_size]
              for i in range(0, len(task_ids), group_size)]

    with tile.TileContext(nc) as tc:
        with (
            tc.tile_pool(name="const", bufs=1) as cpool,
            tc.tile_pool(name="persist", bufs=1) as ppool,
            tc.tile_pool(name="qk", bufs=4) as qkpool,
            tc.tile_pool(name="ot", bufs=4) as otpool,
            tc.tile_pool(name="psA", bufs=4, space="PSUM") as psA,
            tc.tile_pool(name="psB", bufs=2, space="PSUM") as psB,
        ):
            ident = cpool.tile([P, P], f32, tag="ident")
            make_identity(nc, ident[:])
            # maskT0[kk, z] = 0 where z >= kk + 384 else NEG (transposed
            # orientation: partition = k, free = q).  Diagonal k-tile ktr of
            # a q-chunk uses the view at z0 = 384 - 128*ktr:
            # maskT0[kk, z0+n] = 0 iff n >= kk + 128*ktr.
            maskT0 = cpool.tile([P, 896], f32, tag="maskT0")
            nc.gpsimd.memset(maskT0[:], 0.0)
            nc.gpsimd.affine_select(
                out=maskT0[:], in_=maskT0[:],
                compare_op=ALU.is_ge, fill=NEG,
                base=-384, channel_multiplier=-1, pattern=[[1, 896]],
            )

            def maskT_r(ktr):
                return maskT0[:, 384 - 128 * ktr:384 - 128 * ktr + 512]

            # all-ones stationary: den matmul out[m,n] = sum_k pt[k,n] for
            # every m, i.e. the denominator row broadcast to all partitions.
            ones32 = cpool.tile([P, P], f32, tag="ones32")
            nc.gpsimd.memset(ones32[:], 1.0)
            ones = cpool.tile([P, P], mm_dt, tag="ones")
            nc.vector.tensor_copy(ones[:], ones32[:])

            for rep in range(iters):
              for grp in groups:
                ntt = len(grp)
                # per-task contiguous Q^T / K^T: qt_c[d, q] with the raw-view
                # interleave (q = i*16 + j) materialized in memory, so matmul
                # RHS/LHS APs are plain slices (single free dim).
                qt_cs = [qkpool.tile([P, S], mm_dt, tag="qtc", name=f"qt{ti}")
                         for ti in range(ntt)]
                kt_cs = [qkpool.tile([P, S], mm_dt, tag="ktc", name=f"kt{ti}")
                         for ti in range(ntt)]
                # vt_c[d, k] = V^T per task, same interleaved-contiguous form
                vt_cs = [qkpool.tile([P, S], f32, tag="vtc", name=f"vt{ti}")
                         for ti in range(ntt)]

                # ---------------- QKV phase ----------------
                with (
                    tc.tile_pool(name="qkvph", bufs=1) as qpool,
                    tc.tile_pool(name="qkvst", bufs=2) as qspool,
                ):
                    # at_all[kk, kc, ti*128+m] = x[task ti row m, kc*128+kk]
                    at_all = qpool.tile([P, 16, ntt * P], mm_dt, tag="at_all")
                    for ti, gi in enumerate(grp):
                        for half in range(2):
                            a_sb = qspool.tile([P, E // 2], f32, tag="a")
                            nc.scalar.dma_start(
                                a_sb[:],
                                xs.ap()[gi * P:(gi + 1) * P,
                                        half * (E // 2):(half + 1) * (E // 2)])
                            for tg in range(2):
                                tp = psA.tile([P, 512], f32, tag="mm512")
                                for sb in range(4):
                                    kc = tg * 4 + sb
                                    nc.tensor.transpose(
                                        tp[:, sb * P:(sb + 1) * P],
                                        a_sb[:, kc * P:(kc + 1) * P],
                                        ident[:])
                                nc.vector.tensor_copy(
                                    at_all[:, half * 8 + tg * 4:
                                           half * 8 + (tg + 1) * 4,
                                           ti * P:(ti + 1) * P],
                                    tp[:].rearrange("p (s m) -> p s m", s=4))

                    for cbp in range(24):
                        wq = qspool.tile([P, 16, 2 * P], mm_dt, tag="wq")
                        nc.sync.dma_start(
                            wq[:], wview(
                                wqkv_v[:, :, cbp * 2 * P:(cbp + 1) * 2 * P]))
                        for half in range(2):
                            cb = cbp * 2 + half
                            ps = psA.tile([P, ntt * P], f32, tag="mm512")
                            for kc in range(16):
                                nc.tensor.matmul(
                                    ps[:],
                                    wq[:, kc, half * P:(half + 1) * P],
                                    at_all[:, kc, :],
                                    start=(kc == 0), stop=(kc == 15))
                            dsts = (qt_cs, kt_cs, vt_cs)[cb // 16]
                            j = cb % 16
                            for ti in range(ntt):
                                nc.vector.tensor_copy(
                                    dsts[ti].rearrange(
                                        "d (i j) -> d i j", j=16)[:, :, j],
                                    ps[:, ti * P:(ti + 1) * P])

                # ---------------- attention phase (per task) ----------------
                ots = []
                with (
                    tc.tile_pool(name="attw", bufs=4) as awpool,
                    tc.tile_pool(name="vn", bufs=2) as vnpool,
                ):
                    for ti, gi in enumerate(grp):
                        # V natural tiles: vnat[kk, kt, d] = V[kt*128+kk, d]
                        vnat = vnpool.tile([P, 16, P], mm_dt, tag="vnat")
                        for ktg in range(4):
                            tp = psA.tile([P, 512], f32, tag="mm512")
                            for sb in range(4):
                                kt = ktg * 4 + sb
                                nc.tensor.transpose(
                                    tp[:, sb * P:(sb + 1) * P],
                                    vt_cs[ti][:, kt * P:(kt + 1) * P],
                                    ident[:])
                            nc.vector.tensor_copy(
                                vnat[:, ktg * 4:(ktg + 1) * 4, :].rearrange(
                                    "p s d -> p (s d)"), tp[:])

                        ot = otpool.tile([P, 16, P], mm_dt, tag="ot")  # O^T
                        ots.append(ot)
                        for qc in range(4):
                            # q-chunk of 512; P^T tiles computed directly as
                            # S^T = K @ Q^T (no PE transposes of P needed).
                            rhsq = qt_cs[ti][:, qc * 512:(qc + 1) * 512]
                            ot_ps = psB.tile([P, 512], f32, tag="otacc")
                            den_ps = psB.tile([P, 512], f32, tag="denacc")
                            nkt = qc * 4 + 4
                            for kt in range(nkt):
                                lhsTk = kt_cs[ti][:, kt * P:(kt + 1) * P]
                                s_ps = psA.tile([P, 512], f32, tag="mm512")
                                nc.tensor.matmul(s_ps[:], lhsTk, rhsq,
                                                 start=True, stop=True)
                                if kt >= qc * 4:  # diagonal: additive mask
                                    nc.vector.tensor_tensor(
                                        s_ps[:], s_ps[:],
                                        maskT_r(kt - qc * 4), ALU.add)
                                pt = awpool.tile([P, 512], mm_dt, tag="pt")
                                nc.scalar.activation(
                                    pt[:], s_ps[:], AF.Exp, bias=1.0,
                                    scale=SCALE)
                                nc.tensor.matmul(
                                    ot_ps[:], vnat[:, kt, :], pt[:],
                                    start=(kt == 0), stop=(kt == nkt - 1))
                                nc.tensor.matmul(
                                    den_ps[:], ones[:], pt[:],
                                    start=(kt == 0), stop=(kt == nkt - 1))
                            rec = awpool.tile([P, 512], f32, tag="rec")
                            nc.vector.reciprocal(rec[:], den_ps[:])
                            nc.vector.tensor_tensor(
                                ot[:, qc * 4:(qc + 1) * 4, :].rearrange(
                                    "p s d -> p (s d)"),
                                ot_ps[:], rec[:], ALU.mult)

                # ---------------- output projection ----------------
                with tc.tile_pool(name="oproj", bufs=2) as opool:
                    for nch in range(4):
                        wo = opool.tile([P, 16, 512], mm_dt, tag="wo")
                        nc.sync.dma_start(
                            wo[:], wview(
                                wout_v[:, :, nch * 512:(nch + 1) * 512]))
                        for ti, gi in enumerate(grp):
                            lt = ots[ti].rearrange(
                                "d qt (i j) -> d qt i j", j=16)
                            ps = psA.tile([P, 512], f32, tag="mm512")
                            for cc in range(16):
                                nc.tensor.matmul(
                                    ps[:], lt[:, :, :, cc],
                                    wo[:, cc, :],
                                    start=(cc == 0), stop=(cc == 15))
                            osb = opool.tile([P, 512], f32, tag="osb")
                            nc.vector.tensor_copy(osb[:], ps[:])
                            nc.scalar.dma_start(
                                out.ap()[gi * P:(gi + 1) * P,
                                         nch * 512:(nch + 1) * 512], osb[:])
    nc.compile()
    return nc


def get_nc(mm="float32r"):
    if mm not in _NC_CACHE:
        _NC_CACHE[mm] = build_nc(mm=mm)
    return _NC_CACHE[mm]


def kernel(x, w_qkv, w_out):
    from concourse.bass_utils import run_bass_kernel_spmd

    x = np.ascontiguousarray(np.asarray(x, dtype=np.float32))
    w_qkv = np.ascontiguousarray(np.asarray(w_qkv, dtype=np.float32))
    w_out = np.ascontiguousarray(np.asarray(w_out, dtype=np.float32))
    nc = get_nc()
    xf = x.reshape(B * S, E)
    in_maps = [
        {"xs": np.ascontiguousarray(xf[c * ROWS:(c + 1) * ROWS]),
         "wqkv": w_qkv, "wout": w_out}
        for c in range(NCORES)
    ]
    res = run_bass_kernel_spmd(nc, in_maps, core_ids=list(range(NCORES)))
    outs = [res.results[c]["out"] for c in range(NCORES)]
    return np.concatenate(outs, axis=0).reshape(B, S, E).astype(np.float32)



# revision 4
# speedup vs baseline: 1.4141x; 1.4141x over previous
"""Causal attention kernel for trn2, sharded over 8 NeuronCores.

Problem (B=4, S=2048, E=2048, H=16, D=128), fp32 in/out:
    qkv = x @ w_qkv; q,k,v = split(qkv)
    q,k,v reshaped (B,S,E)->(B,H,S,D) as a RAW view (no transpose), i.e.
    per (b,h): Q_h = rows [h*128,(h+1)*128) of q[b] reinterpreted [S,D].
    o = softmax(QK^T/sqrt(D) + causal(+1/-10000)) @ V, inverse raw view,
    out = o @ w_out.

Because the raw view maps head h to a contiguous block of 128 sequence
rows, the whole computation splits into B*H = 64 independent tasks, each
touching only x[b, h*128:(h+1)*128, :] and producing
out[b, h*128:(h+1)*128, :].  Core c gets 8 tasks = rows
[c*1024,(c+1)*1024) of x.reshape(B*S, E).  No collectives.

All matmuls run in bf16 (inputs converted host-side, x pre-transposed
host-side into the PE-stationary layout); accumulation stays fp32 in
PSUM.  Tolerance is 2e-2 max-rel; bf16 lands ~2e-3.
"""

import numpy as np

B, S, E = 4, 2048, 2048
H, D, P = 16, 128, 128
NCORES = 8
ROWS = B * S // NCORES   # 1024 rows per core = 8 tasks of 128 rows
NGRP = 2                 # task groups per core
NTT = 4                  # tasks per group
SCALE = float(1.0 / np.sqrt(D))
NEG = -1.0e9  # pre-scale additive mask; exp underflows to exactly 0.0

_NC_CACHE = {}


def build_nc():
    import concourse.mybir as mybir
    import concourse.tile as tile
    from concourse import bacc
    from concourse.masks import make_identity

    f32 = mybir.dt.float32
    bf16 = mybir.dt.bfloat16
    AF = mybir.ActivationFunctionType
    ALU = mybir.AluOpType

    nc = bacc.Bacc("TRN2", target_bir_lowering=False, debug=False,
                   num_devices=NCORES)
    # xt[kk, g, kc, ti*128+m] = x[row g*512+ti*128+m, kc*128+kk] (host-
    # pretransposed, bf16): DMAs straight into the matmul-ready layout.
    xt = nc.dram_tensor("xt", [P, NGRP * 16 * NTT * P], bf16,
                        kind="ExternalInput")
    wqkv = nc.dram_tensor("wqkv", [E, 3 * E], bf16, kind="ExternalInput")
    wout = nc.dram_tensor("wout", [E, E], bf16, kind="ExternalInput")
    out = nc.dram_tensor("out", [ROWS, E], f32, kind="ExternalOutput")

    xt_v = xt.ap().rearrange("p (g k) -> p g k", g=NGRP)      # [128,2,8192]
    wqkv_v = wqkv.ap().rearrange("(ko p) c -> p ko c", p=P)   # [128,16,6144]
    wout_v = wout.ap().rearrange("(co p) n -> p co n", p=P)   # [128,16,2048]

    with tile.TileContext(nc) as tc:
        with (
            tc.tile_pool(name="const", bufs=1) as cpool,
            tc.tile_pool(name="atp", bufs=2) as atpool,
            tc.tile_pool(name="qk", bufs=4) as qkpool,
            tc.tile_pool(name="ot", bufs=4) as otpool,
            tc.tile_pool(name="wq", bufs=2) as wqpool,
            tc.tile_pool(name="wo", bufs=2) as wopool,
            tc.tile_pool(name="attw", bufs=4) as awpool,
            tc.tile_pool(name="vn", bufs=2) as vnpool,
            tc.tile_pool(name="osb", bufs=2) as ospool,
            tc.tile_pool(name="psA", bufs=4, space="PSUM") as psA,
            tc.tile_pool(name="psB", bufs=2, space="PSUM") as psB,
        ):
            ident = cpool.tile([P, P], bf16, tag="ident")
            make_identity(nc, ident[:])
            # maskT0[kk, z] = 0 where z >= kk + 384 else NEG (transposed
            # orientation: partition = k, free = q).  Diagonal k-tile ktr of
            # a q-chunk uses the view at z0 = 384 - 128*ktr:
            # maskT0[kk, z0+n] = 0 iff n >= kk + 128*ktr.
            maskT0 = cpool.tile([P, 896], f32, tag="maskT0")
            nc.gpsimd.memset(maskT0[:], 0.0)
            nc.gpsimd.affine_select(
                out=maskT0[:], in_=maskT0[:],
                compare_op=ALU.is_ge, fill=NEG,
                base=-384, channel_multiplier=-1, pattern=[[1, 896]],
            )

            def maskT_r(ktr):
                return maskT0[:, 384 - 128 * ktr:384 - 128 * ktr + 512]

            # all-ones stationary: den matmul out[m,n] = sum_k pt[k,n] for
            # every m, i.e. the denominator row broadcast to all partitions.
            ones = cpool.tile([P, P], bf16, tag="ones")
            nc.gpsimd.memset(ones[:], 1.0)

            at_g = [atpool.tile([P, 16, NTT * P], bf16, tag="at_all",
                                name=f"at{g}") for g in range(NGRP)]
            nc.sync.dma_start(at_g[0][:].rearrange("p a b -> p (a b)"),
                              xt_v[:, 0, :])

            for g in range(NGRP):
                qt_cs = [qkpool.tile([P, S], bf16, tag="qtc", name=f"qt{ti}")
                         for ti in range(NTT)]
                kt_cs = [qkpool.tile([P, S], bf16, tag="ktc", name=f"kt{ti}")
                         for ti in range(NTT)]
                vt_cs = [qkpool.tile([P, S], bf16, tag="vtc", name=f"vt{ti}")
                         for ti in range(NTT)]

                # ---------------- QKV phase ----------------
                for cbp in range(24):
                    wq = wqpool.tile([P, 16, 2 * P], bf16, tag="wq")
                    nc.sync.dma_start(
                        wq[:], wqkv_v[:, :, cbp * 2 * P:(cbp + 1) * 2 * P])
                    for half in range(2):
                        cb = cbp * 2 + half
                        ps = psA.tile([P, NTT * P], f32, tag="mm512")
                        for kc in range(16):
                            nc.tensor.matmul(
                                ps[:],
                                wq[:, kc, half * P:(half + 1) * P],
                                at_g[g][:, kc, :],
                                start=(kc == 0), stop=(kc == 15))
                        dsts = (qt_cs, kt_cs, vt_cs)[cb // 16]
                        j = cb % 16
                        for ti in range(NTT):
                            nc.vector.tensor_copy(
                                dsts[ti].rearrange(
                                    "d (i j) -> d i j", j=16)[:, :, j],
                                ps[:, ti * P:(ti + 1) * P])

                # ---------------- attention phase (per task) ----------------
                if g + 1 < NGRP:
                    nc.sync.dma_start(
                        at_g[g + 1][:].rearrange("p a b -> p (a b)"),
                        xt_v[:, g + 1, :])
                wos = [wopool.tile([P, 16, 512], bf16, tag="wo",
                                   name=f"wo{nch}") for nch in range(4)]
                for nch in range(2):
                    nc.sync.dma_start(
                        wos[nch][:],
                        wout_v[:, :, nch * 512:(nch + 1) * 512])

                ots = []
                for ti in range(NTT):
                    # V natural tiles: vnat[kk, kt, d] = V[kt*128+kk, d]
                    vnat = vnpool.tile([P, 16, P], bf16, tag="vnat")
                    for ktg in range(4):
                        tp = psA.tile([P, 512], bf16, tag="mm512",
                                      padded_shape=[P, 1024])
                        for sb in range(4):
                            kt = ktg * 4 + sb
                            nc.tensor.transpose(
                                tp[:, sb * P:(sb + 1) * P],
                                vt_cs[ti][:, kt * P:(kt + 1) * P],
                                ident[:])
                        nc.vector.tensor_copy(
                            vnat[:, ktg * 4:(ktg + 1) * 4, :].rearrange(
                                "p s d -> p (s d)"), tp[:])

                    ot = otpool.tile([P, 16, P], bf16, tag="ot")  # O^T
                    ots.append(ot)
                    for qc in range(4):
                        # q-chunk of 512; P^T tiles computed directly as
                        # S^T = K @ Q^T (no PE transposes of P needed).
                        rhsq = qt_cs[ti][:, qc * 512:(qc + 1) * 512]
                        ot_ps = psB.tile([P, 512], f32, tag="otacc")
                        den_ps = psB.tile([P, 512], f32, tag="denacc")
                        nkt = qc * 4 + 4
                        for kt in range(nkt):
                            lhsTk = kt_cs[ti][:, kt * P:(kt + 1) * P]
                            s_ps = psA.tile([P, 512], f32, tag="mm512")
                            nc.tensor.matmul(s_ps[:], lhsTk, rhsq,
                                             start=True, stop=True)
                            if kt >= qc * 4:  # diagonal: additive mask
                                nc.vector.tensor_tensor(
                                    s_ps[:], s_ps[:],
                                    maskT_r(kt - qc * 4), ALU.add)
                            pt = awpool.tile([P, 512], bf16, tag="pt")
                            nc.scalar.activation(
                                pt[:], s_ps[:], AF.Exp, bias=1.0,
                                scale=SCALE)
                            nc.tensor.matmul(
                                ot_ps[:], vnat[:, kt, :], pt[:],
                                start=(kt == 0), stop=(kt == nkt - 1))
                            nc.tensor.matmul(
                                den_ps[:], ones[:], pt[:],
                                start=(kt == 0), stop=(kt == nkt - 1))
                        rec = awpool.tile([P, 512], f32, tag="rec")
                        nc.vector.reciprocal_approx_fast(
                            out=rec[:], in_=den_ps[:])
                        nc.vector.tensor_tensor(
                            ot[:, qc * 4:(qc + 1) * 4, :].rearrange(
                                "p s d -> p (s d)"),
                            ot_ps[:], rec[:], ALU.mult)

                # ---------------- output projection ----------------
                for nch in range(2, 4):
                    nc.sync.dma_start(
                        wos[nch][:],
                        wout_v[:, :, nch * 512:(nch + 1) * 512])
                for nch in range(4):
                    for ti in range(NTT):
                        lt = ots[ti].rearrange("d qt (i j) -> d qt i j", j=16)
                        ps = psA.tile([P, 512], f32, tag="mm512")
                        for cc in range(16):
                            nc.tensor.matmul(
                                ps[:], lt[:, :, :, cc],
                                wos[nch][:, cc, :],
                                start=(cc == 0), stop=(cc == 15))
                        osb = ospool.tile([P, 512], f32, tag="osb")
                        nc.vector.tensor_copy(osb[:], ps[:])
                        nc.scalar.dma_start(
                            out.ap()[(g * NTT + ti) * P:
                                     (g * NTT + ti + 1) * P,
                                     nch * 512:(nch + 1) * 512], osb[:])
    nc.compile()
    return nc


def get_nc():
    if "nc" not in _NC_CACHE:
        _NC_CACHE["nc"] = build_nc()
    return _NC_CACHE["nc"]


def make_in_maps(x, w_qkv, w_out):
    import ml_dtypes

    bf = ml_dtypes.bfloat16
    xf = np.ascontiguousarray(np.asarray(x, dtype=np.float32)).reshape(
        B * S, E).astype(bf)
    wqkv_b = np.ascontiguousarray(
        np.asarray(w_qkv, dtype=np.float32).astype(bf))
    wout_b = np.ascontiguousarray(
        np.asarray(w_out, dtype=np.float32).astype(bf))
    # xt[c][kk, g, kc, ti, m] = x[c*1024 + g*512 + ti*128 + m, kc*128 + kk]
    xa = xf.reshape(NCORES, NGRP, NTT, P, 16, P).transpose(0, 5, 1, 4, 2, 3)
    in_maps = [
        {"xt": np.ascontiguousarray(xa[c]).reshape(P, NGRP * 16 * NTT * P),
         "wqkv": wqkv_b, "wout": wout_b}
        for c in range(NCORES)
    ]
    return in_maps


def kernel(x, w_qkv, w_out):
    from concourse.bass_utils import run_bass_kernel_spmd

    nc = get_nc()
    in_maps = make_in_maps(x, w_qkv, w_out)
    res = run_bass_kernel_spmd(nc, in_maps, core_ids=list(range(NCORES)))
    outs = [res.results[c]["out"] for c in range(NCORES)]
    return np.concatenate(outs, axis=0).reshape(B, S, E).astype(np.float32)
